# revision 19
# baseline (speedup 1.0000x reference)
"""MultiHeadAttention TRN2 kernel: data-parallel over batch (8 cores, 1 batch elem each).

Per-core schedule ("T-layout": every contraction keeps its reduction dim on SBUF
partitions, so no on-device transposes are needed):
  per head h:
    qT[f,s] = Wq[h].T @ x[b].T   (contract e)   kT likewise
    v[t,f]  = x[b] @ Wv[h]       (contract e)
    scT[t,s] = k @ qT            (contract f);  expE = exp(scT/sqrt(E)) fused on ACT
    denom[s] = ones.T @ expE     (contract t, broadcast to all partitions); recip on DVE
    oT[f,s] = v.T @ expE         (contract t);  normalized via tensor_mul -> bf16
  out[s,e] = sum_hf oT[hf].T @ Wo[hf]  (32-step PSUM accumulation)
"""

import math
import os
from contextlib import ExitStack

import numpy as np
import ml_dtypes

from concourse import bacc, bass, bass_utils, tile

mybir = bass.mybir
BF16 = mybir.dt.bfloat16
F32 = mybir.dt.float32
FP8 = mybir.dt.float8e4
AF = mybir.ActivationFunctionType

B, S, E, H = 8, 1024, 512, 8
ET = E // 128    # 4  chunks of the embedding dim
TT = S // 128    # 8  chunks of the sequence dim
SC = S // 512    # 2  moving-dim chunks of the sequence dim
HF = (H * E) // 128  # 32 chunks of the concat-head dim
SCALE = 1.0 / math.sqrt(E)

_compiled_nc = None
last_exec_time_ns = None


def _emit(ctx, tc, xT_d, wq_d, wk_d, wv_d, wo_d, out_d):
    nc = tc.nc

    const_pool = ctx.enter_context(tc.tile_pool(name="const", bufs=1))
    # bufs=1 serializes head h+1's weight DMA behind head h's last weight
    # read, keeping the gpsimd software DMA queue quiet during the startup
    # window where it would otherwise starve the hw queues feeding Phase A
    w_pool = ctx.enter_context(tc.tile_pool(name="wqkv", bufs=1))
    act_pool = ctx.enter_context(tc.tile_pool(name="acts", bufs=1))
    out_pool = ctx.enter_context(tc.tile_pool(name="outp", bufs=2))
    psum_pool = ctx.enter_context(tc.tile_pool(name="ps", bufs=6, space="PSUM"))

    xT_r = xT_d.rearrange("(et p) s -> p et s", p=128)
    xT_sb = const_pool.tile([128, ET, S], BF16)       # [p=e, et, s]
    wo_sb = const_pool.tile([128, HF, E], BF16)       # [p=f, hf, e]
    ones8_sb = const_pool.tile([128, 2, 128], FP8)
    oT_all = const_pool.tile([128, HF, S], BF16)      # [p=f, hf, s]

    wq_r = wq_d.rearrange("h (et p) f -> h p et f", p=128)
    wk_r = wk_d.rearrange("h (et p) f -> h p et f", p=128)
    wv_r = wv_d.rearrange("h (et p) f -> h p et f", p=128)

    for h in range(H):
        wq_sb = w_pool.tile([128, ET, E], BF16)
        wk_sb = w_pool.tile([128, ET, E], BF16)
        wv_sb = w_pool.tile([128, ET, E], BF16)
        if h == 0:
            # First-needed data on the two fast hardware queues (sync/scalar);
            # everything else on the slower gpsimd software queue.
            # Startup: hw queues carry ONLY what Phase A q needs (wq + xT),
            # split so both queues finish before the bulk gpsimd traffic
            # (wk/wv/wo) starts crawling the HBM fabric at ~11.6us.
            nc.sync.dma_start(wq_sb[:], wq_r[0])
            nc.scalar.dma_start(xT_sb[:, 2:4, 0:512], xT_r[:, 2:4, 0:512])
            nc.sync.dma_start(xT_sb[:, 0:2, 0:512], xT_r[:, 0:2, 0:512])
            nc.scalar.dma_start(xT_sb[:, 2:4, 512:1024], xT_r[:, 2:4, 512:1024])
            nc.sync.dma_start(xT_sb[:, 0:2, 512:1024], xT_r[:, 0:2, 512:1024])
            nc.gpsimd.dma_start(wk_sb[:], wk_r[0])
            nc.gpsimd.dma_start(wv_sb[:], wv_r[0])
            nc.gpsimd.dma_start(wo_sb[:], wo_d.rearrange("(hf p) e -> p hf e", p=128))
            nc.gpsimd.memset(ones8_sb[:], 1.0)
        else:
            nc.gpsimd.dma_start(wq_sb[:], wq_r[h])
            nc.gpsimd.dma_start(wk_sb[:], wk_r[h])
            nc.gpsimd.dma_start(wv_sb[:], wv_r[h])

        qT_sb = act_pool.tile([128, ET, S], BF16)     # [p=f, ft, s]
        kT_sb = act_pool.tile([128, ET, S], BF16)
        v_sb = act_pool.tile([128, TT, E], BF16)      # [p=t, tt, f]
        expE_sb = act_pool.tile([128, TT, S], BF16)   # [p=t, tt, s]
        expE8_sb = act_pool.tile([128, TT, S], FP8)   # fp8 copy for denominator
        recip_sb = act_pool.tile([128, SC, 512], F32)

        # q/k projections -> [f, s]
        for w_sb, dst in ((wq_sb, qT_sb), (wk_sb, kT_sb)):
            for sc in range(SC):
                for ft in range(ET):
                    ps = psum_pool.tile([128, 512], F32)
                    for et in range(ET):
                        nc.tensor.matmul(
                            ps[:],
                            w_sb[:, et, ft * 128:(ft + 1) * 128],
                            xT_sb[:, et, sc * 512:(sc + 1) * 512],
                            start=(et == 0), stop=(et == ET - 1),
                        )
                    nc.scalar.activation(
                        dst[:, ft, sc * 512:(sc + 1) * 512], ps[:], AF.Copy)

        # v projection -> [t, f]
        for tt in range(TT):
            ps = psum_pool.tile([128, 512], F32)
            for et in range(ET):
                nc.tensor.matmul(
                    ps[:],
                    xT_sb[:, et, tt * 128:(tt + 1) * 128],
                    wv_sb[:, et, :],
                    start=(et == 0), stop=(et == ET - 1),
                )
            nc.scalar.activation(v_sb[:, tt, :], ps[:], AF.Copy)

        # scoresT + fused exp(scale*scores), sc-outer so each sc's
        # denominator (fp8 DoubleRow ones-matmul, 2x PE throughput) and
        # reciprocal pipeline into the middle of the phase
        for sc in range(SC):
            for tt in range(TT):
                ps = psum_pool.tile([128, 512], F32)
                for ft in range(ET):
                    nc.tensor.matmul(
                        ps[:],
                        kT_sb[:, ft, tt * 128:(tt + 1) * 128],
                        qT_sb[:, ft, sc * 512:(sc + 1) * 512],
                        start=(ft == 0), stop=(ft == ET - 1),
                    )
                nc.scalar.activation(
                    expE_sb[:, tt, sc * 512:(sc + 1) * 512], ps[:],
                    AF.Exp, scale=SCALE)
                nc.vector.tensor_copy(
                    expE8_sb[:, tt, sc * 512:(sc + 1) * 512],
                    expE_sb[:, tt, sc * 512:(sc + 1) * 512])
            ps = psum_pool.tile([128, 512], F32)
            for tt in range(0, TT, 2):
                nc.tensor.matmul(
                    ps[:], ones8_sb[:, 0:2, :],
                    expE8_sb[:, tt:tt + 2, sc * 512:(sc + 1) * 512],
                    start=(tt == 0), stop=(tt == TT - 2),
                    perf_mode=mybir.MatmulPerfMode.DoubleRow,
                )
            nc.vector.reciprocal(recip_sb[:, sc, :], ps[:])

        # oT = v.T @ expE, normalized; sc-outer so sc0's tensor_muls drain
        # while sc1's reciprocal is still in flight
        for sc in range(SC):
            for ft in range(ET):
                ps = psum_pool.tile([128, 512], F32)
                for tt in range(TT):
                    nc.tensor.matmul(
                        ps[:],
                        v_sb[:, tt, ft * 128:(ft + 1) * 128],
                        expE_sb[:, tt, sc * 512:(sc + 1) * 512],
                        start=(tt == 0), stop=(tt == TT - 1),
                    )
                nc.vector.tensor_mul(
                    oT_all[:, h * ET + ft, sc * 512:(sc + 1) * 512],
                    ps[:], recip_sb[:, sc, :])

    # output projection: out[s, e] = sum_f o_concat[s, f] Wo[f, e]
    out_r = out_d.rearrange("(st p) e -> p st e", p=128)
    for st in range(TT):
        ps = psum_pool.tile([128, 512], F32)
        for hf in range(HF):
            nc.tensor.matmul(
                ps[:],
                oT_all[:, hf, st * 128:(st + 1) * 128],
                wo_sb[:, hf, :],
                start=(hf == 0), stop=(hf == HF - 1),
            )
        o_sb = out_pool.tile([128, 512], F32)
        nc.vector.tensor_copy(o_sb[:], ps[:])
        nc.sync.dma_start(out_r[:, st, :], o_sb[:])


def _build():
    nc = bacc.Bacc("TRN2", target_bir_lowering=False, debug=False,
                   enable_asserts=False, num_devices=B)
    xT_d = nc.dram_tensor("xT", [E, S], BF16, kind="ExternalInput").ap()
    wq_d = nc.dram_tensor("wq", [H, E, E], BF16, kind="ExternalInput").ap()
    wk_d = nc.dram_tensor("wk", [H, E, E], BF16, kind="ExternalInput").ap()
    wv_d = nc.dram_tensor("wv", [H, E, E], BF16, kind="ExternalInput").ap()
    wo_d = nc.dram_tensor("wo", [H * E, E], BF16, kind="ExternalInput").ap()
    out_d = nc.dram_tensor("out", [S, E], F32, kind="ExternalOutput").ap()

    with tile.TileContext(nc) as tc, ExitStack() as ctx:
        _emit(ctx, tc, xT_d, wq_d, wk_d, wv_d, wo_d, out_d)
    nc.compile()
    return nc


def kernel(x, Wq, Wk, Wv, Wo, **_unused_zero_biases):
    global _compiled_nc, last_exec_time_ns
    if _compiled_nc is None:
        _compiled_nc = _build()

    bf = ml_dtypes.bfloat16
    x = np.asarray(x)
    wq_np = np.asarray(Wq).astype(bf)
    wk_np = np.asarray(Wk).astype(bf)
    wv_np = np.asarray(Wv).astype(bf)
    wo_np = np.asarray(Wo).astype(bf)
    in_maps = [
        {"xT": x[b].T.astype(bf), "wq": wq_np, "wk": wk_np,
         "wv": wv_np, "wo": wo_np}
        for b in range(B)
    ]
    trace = bool(int(os.environ.get("KERNEL_TRACE", "0")))
    res = bass_utils.run_bass_kernel_spmd(
        _compiled_nc, in_maps, core_ids=list(range(B)), trace=trace)
    last_exec_time_ns = res.exec_time_ns
    return np.stack([res.results[b]["out"] for b in range(B)], axis=0)


# revision 27
# speedup vs baseline: 1.0082x; 1.0082x over previous
"""MultiHeadAttention TRN2 kernel: data-parallel over batch (8 cores, 1 batch elem each).

Per-core schedule ("T-layout": every contraction keeps its reduction dim on SBUF
partitions, so no on-device transposes are needed):
  per head h:
    qT[f,s] = Wq[h].T @ x[b].T   (contract e)   kT likewise
    v[t,f]  = x[b] @ Wv[h]       (contract e)
    scT[t,s] = k @ qT            (contract f);  expE = exp(scT/sqrt(E)) fused on ACT
    denom[s] = ones.T @ expE     (contract t, broadcast to all partitions); recip on DVE
    oT[f,s] = v.T @ expE         (contract t);  normalized via tensor_mul -> bf16
  out[s,e] = sum_hf oT[hf].T @ Wo[hf]  (32-step PSUM accumulation)
"""

import math
import os
from contextlib import ExitStack

import numpy as np
import ml_dtypes

from concourse import bacc, bass, bass_utils, tile

mybir = bass.mybir
BF16 = mybir.dt.bfloat16
F32 = mybir.dt.float32
FP8 = mybir.dt.float8e4
AF = mybir.ActivationFunctionType

B, S, E, H = 8, 1024, 512, 8
ET = E // 128    # 4  chunks of the embedding dim
TT = S // 128    # 8  chunks of the sequence dim
SC = S // 512    # 2  moving-dim chunks of the sequence dim
HF = (H * E) // 128  # 32 chunks of the concat-head dim
SCALE = 1.0 / math.sqrt(E)

_compiled_nc = None
last_exec_time_ns = None


def _emit(ctx, tc, wx_d, wq_d, wk_d, wv_d, wo_d, out_d):
    nc = tc.nc

    const_pool = ctx.enter_context(tc.tile_pool(name="const", bufs=1))
    # bufs=1 serializes head h+1's weight DMA behind head h's last weight
    # read, keeping the gpsimd software DMA queue quiet during the startup
    # window where it would otherwise starve the hw queues feeding Phase A
    w_pool = ctx.enter_context(tc.tile_pool(name="wqkv", bufs=1))
    act_pool = ctx.enter_context(tc.tile_pool(name="acts", bufs=1))
    out_pool = ctx.enter_context(tc.tile_pool(name="outp", bufs=2))
    psum_pool = ctx.enter_context(tc.tile_pool(name="ps", bufs=6, space="PSUM"))

    # wx = [xT cols 0:512 | Wq[0] | xT cols 512:1024] packed host-side so all
    # of Phase A's first working set arrives via ONE dma per hw queue (only a
    # hw queue's first dma streams at full rate; later ones crawl)
    wx_r = wx_d.rearrange("(et p) c -> p et c", p=128)
    wx_sb = const_pool.tile([128, ET, 1536], BF16)    # [p=e, et, c]
    wo_sb = const_pool.tile([128, HF, E], BF16)       # [p=f, hf, e]
    ones8_sb = const_pool.tile([128, 2, 128], FP8)
    oT_all = const_pool.tile([128, HF, S], BF16)      # [p=f, hf, s]

    wq_r = wq_d.rearrange("h (et p) f -> h p et f", p=128)
    wk_r = wk_d.rearrange("h (et p) f -> h p et f", p=128)
    wv_r = wv_d.rearrange("h (et p) f -> h p et f", p=128)

    for h in range(H):
        wq_sb = w_pool.tile([128, ET, E], BF16)
        wk_sb = w_pool.tile([128, ET, E], BF16)
        wv_sb = w_pool.tile([128, ET, E], BF16)
        if h == 0:
            # Split point 832: sync (first packet ~8.8us) carries xT half1 +
            # wq cols 0:320; scalar (first packet ~10.5us) the rest, landing
            # before the ft2 q-group and the sc1 matmuls need them. Only a hw
            # queue's FIRST dma streams fast, so exactly one dma per queue.
            nc.sync.dma_start(wx_sb[:, :, 0:832], wx_r[:, :, 0:832])
            nc.scalar.dma_start(wx_sb[:, :, 832:1536], wx_r[:, :, 832:1536])
            nc.gpsimd.dma_start(wk_sb[:], wk_r[0])
            nc.gpsimd.dma_start(wv_sb[:], wv_r[0])
            nc.gpsimd.dma_start(wo_sb[:], wo_d.rearrange("(hf p) e -> p hf e", p=128))
            nc.gpsimd.memset(ones8_sb[:], 1.0)
        else:
            nc.gpsimd.dma_start(wq_sb[:], wq_r[h])
            nc.gpsimd.dma_start(wk_sb[:], wk_r[h])
            nc.gpsimd.dma_start(wv_sb[:], wv_r[h])

        qT_sb = act_pool.tile([128, ET, S], BF16)     # [p=f, ft, s]
        kT_sb = act_pool.tile([128, ET, S], BF16)
        v_sb = act_pool.tile([128, TT, E], BF16)      # [p=t, tt, f]
        expE_sb = act_pool.tile([128, TT, S], BF16)   # [p=t, tt, s]
        expE8_sb = act_pool.tile([128, TT, S], FP8)   # fp8 copy for denominator
        recip_sb = act_pool.tile([128, SC, 512], F32)

        # q/k projections -> [f, s]; xT lives in wx cols 0:512 (sc0) and
        # 1024:1536 (sc1); head-0 Wq is packed into wx cols 512:1024
        qw = (wx_sb, 512) if h == 0 else (wq_sb, 0)
        for (w_t, w_off), dst in ((qw, qT_sb), ((wk_sb, 0), kT_sb)):
            for sc in range(SC):
                for ft in range(ET):
                    ps = psum_pool.tile([128, 512], F32)
                    for et in range(ET):
                        nc.tensor.matmul(
                            ps[:],
                            w_t[:, et, w_off + ft * 128:w_off + (ft + 1) * 128],
                            wx_sb[:, et, sc * 1024:sc * 1024 + 512],
                            start=(et == 0), stop=(et == ET - 1),
                        )
                    nc.scalar.activation(
                        dst[:, ft, sc * 512:(sc + 1) * 512], ps[:], AF.Copy)

        # v projection -> [t, f]
        for tt in range(TT):
            xo = tt * 128 if tt < 4 else 1024 + (tt - 4) * 128
            ps = psum_pool.tile([128, 512], F32)
            for et in range(ET):
                nc.tensor.matmul(
                    ps[:],
                    wx_sb[:, et, xo:xo + 128],
                    wv_sb[:, et, :],
                    start=(et == 0), stop=(et == ET - 1),
                )
            nc.scalar.activation(v_sb[:, tt, :], ps[:], AF.Copy)

        # scoresT + fused exp(scale*scores), sc-outer so each sc's
        # denominator (fp8 DoubleRow ones-matmul, 2x PE throughput) and
        # reciprocal pipeline into the middle of the phase
        for sc in range(SC):
            for tt in range(TT):
                ps = psum_pool.tile([128, 512], F32)
                for ft in range(ET):
                    nc.tensor.matmul(
                        ps[:],
                        kT_sb[:, ft, tt * 128:(tt + 1) * 128],
                        qT_sb[:, ft, sc * 512:(sc + 1) * 512],
                        start=(ft == 0), stop=(ft == ET - 1),
                    )
                nc.scalar.activation(
                    expE_sb[:, tt, sc * 512:(sc + 1) * 512], ps[:],
                    AF.Exp, scale=SCALE)
                nc.vector.tensor_copy(
                    expE8_sb[:, tt, sc * 512:(sc + 1) * 512],
                    expE_sb[:, tt, sc * 512:(sc + 1) * 512])
            ps = psum_pool.tile([128, 512], F32)
            for tt in range(0, TT, 2):
                nc.tensor.matmul(
                    ps[:], ones8_sb[:, 0:2, :],
                    expE8_sb[:, tt:tt + 2, sc * 512:(sc + 1) * 512],
                    start=(tt == 0), stop=(tt == TT - 2),
                    perf_mode=mybir.MatmulPerfMode.DoubleRow,
                )
            nc.vector.reciprocal(recip_sb[:, sc, :], ps[:])

        # oT = v.T @ expE, normalized; sc-outer so sc0's tensor_muls drain
        # while sc1's reciprocal is still in flight
        for sc in range(SC):
            for ft in range(ET):
                ps = psum_pool.tile([128, 512], F32)
                for tt in range(TT):
                    nc.tensor.matmul(
                        ps[:],
                        v_sb[:, tt, ft * 128:(ft + 1) * 128],
                        expE_sb[:, tt, sc * 512:(sc + 1) * 512],
                        start=(tt == 0), stop=(tt == TT - 1),
                    )
                nc.vector.tensor_mul(
                    oT_all[:, h * ET + ft, sc * 512:(sc + 1) * 512],
                    ps[:], recip_sb[:, sc, :])

    # output projection: out[s, e] = sum_f o_concat[s, f] Wo[f, e]
    out_r = out_d.rearrange("(st p) e -> p st e", p=128)
    for st in range(TT):
        ps = psum_pool.tile([128, 512], F32)
        for hf in range(HF):
            nc.tensor.matmul(
                ps[:],
                oT_all[:, hf, st * 128:(st + 1) * 128],
                wo_sb[:, hf, :],
                start=(hf == 0), stop=(hf == HF - 1),
            )
        o_sb = out_pool.tile([128, 512], F32)
        nc.vector.tensor_copy(o_sb[:], ps[:])
        nc.sync.dma_start(out_r[:, st, :], o_sb[:])


def _build():
    nc = bacc.Bacc("TRN2", target_bir_lowering=False, debug=False,
                   enable_asserts=False, num_devices=B)
    wx_d = nc.dram_tensor("wx", [E, 1536], BF16, kind="ExternalInput").ap()
    wq_d = nc.dram_tensor("wq", [H, E, E], BF16, kind="ExternalInput").ap()
    wk_d = nc.dram_tensor("wk", [H, E, E], BF16, kind="ExternalInput").ap()
    wv_d = nc.dram_tensor("wv", [H, E, E], BF16, kind="ExternalInput").ap()
    wo_d = nc.dram_tensor("wo", [H * E, E], BF16, kind="ExternalInput").ap()
    out_d = nc.dram_tensor("out", [S, E], F32, kind="ExternalOutput").ap()

    with tile.TileContext(nc) as tc, ExitStack() as ctx:
        _emit(ctx, tc, wx_d, wq_d, wk_d, wv_d, wo_d, out_d)
    nc.compile()
    return nc


def kernel(x, Wq, Wk, Wv, Wo, **_unused_zero_biases):
    global _compiled_nc, last_exec_time_ns
    if _compiled_nc is None:
        _compiled_nc = _build()

    bf = ml_dtypes.bfloat16
    x = np.asarray(x)
    wq_np = np.asarray(Wq).astype(bf)
    wk_np = np.asarray(Wk).astype(bf)
    wv_np = np.asarray(Wv).astype(bf)
    wo_np = np.asarray(Wo).astype(bf)
    in_maps = []
    for b in range(B):
        xTb = x[b].T.astype(bf)
        wx = np.concatenate([xTb[:, 0:512], wq_np[0], xTb[:, 512:1024]], axis=1)
        in_maps.append({"wx": wx, "wq": wq_np, "wk": wk_np,
                        "wv": wv_np, "wo": wo_np})
    trace = bool(int(os.environ.get("KERNEL_TRACE", "0")))
    res = bass_utils.run_bass_kernel_spmd(
        _compiled_nc, in_maps, core_ids=list(range(B)), trace=trace)
    last_exec_time_ns = res.exec_time_ns
    return np.stack([res.results[b]["out"] for b in range(B)], axis=0)


# revision 29
# speedup vs baseline: 1.0133x; 1.0051x over previous
"""MultiHeadAttention TRN2 kernel: data-parallel over batch (8 cores, 1 batch elem each).

Per-core schedule ("T-layout": every contraction keeps its reduction dim on SBUF
partitions, so no on-device transposes are needed):
  per head h:
    qT[f,s] = Wq[h].T @ x[b].T   (contract e)   kT likewise
    v[t,f]  = x[b] @ Wv[h]       (contract e)
    scT[t,s] = k @ qT            (contract f);  expE = exp(scT/sqrt(E)) fused on ACT
    denom[s] = ones.T @ expE     (contract t, broadcast to all partitions); recip on DVE
    oT[f,s] = v.T @ expE         (contract t);  normalized via tensor_mul -> bf16
  out[s,e] = sum_hf oT[hf].T @ Wo[hf]  (32-step PSUM accumulation)
"""

import math
import os
from contextlib import ExitStack

import numpy as np
import ml_dtypes

from concourse import bacc, bass, bass_utils, tile

mybir = bass.mybir
BF16 = mybir.dt.bfloat16
F32 = mybir.dt.float32
FP8 = mybir.dt.float8e4
AF = mybir.ActivationFunctionType

B, S, E, H = 8, 1024, 512, 8
ET = E // 128    # 4  chunks of the embedding dim
TT = S // 128    # 8  chunks of the sequence dim
SC = S // 512    # 2  moving-dim chunks of the sequence dim
HF = (H * E) // 128  # 32 chunks of the concat-head dim
SCALE = 1.0 / math.sqrt(E)

_compiled_nc = None
last_exec_time_ns = None


def _emit(ctx, tc, wx_d, wq_d, wk_d, wv_d, wo_d, out_d):
    nc = tc.nc

    const_pool = ctx.enter_context(tc.tile_pool(name="const", bufs=1))
    # bufs=1 serializes head h+1's weight DMA behind head h's last weight
    # read, keeping the gpsimd software DMA queue quiet during the startup
    # window where it would otherwise starve the hw queues feeding Phase A
    w_pool = ctx.enter_context(tc.tile_pool(name="wqkv", bufs=1))
    act_pool = ctx.enter_context(tc.tile_pool(name="acts", bufs=1))
    out_pool = ctx.enter_context(tc.tile_pool(name="outp", bufs=2))
    psum_pool = ctx.enter_context(tc.tile_pool(name="ps", bufs=6, space="PSUM"))

    # wx = [xT cols 0:512 | Wq[0] | xT cols 512:1024] packed host-side so all
    # of Phase A's first working set arrives via ONE dma per hw queue (only a
    # hw queue's first dma streams at full rate; later ones crawl)
    wx_r = wx_d.rearrange("(et p) c -> p et c", p=128)
    wx_sb = const_pool.tile([128, ET, 1536], BF16)    # [p=e, et, c]
    wo_sb = const_pool.tile([128, HF, E], BF16)       # [p=f, hf, e]
    ones8_sb = const_pool.tile([128, 2, 128], FP8)
    oT_all = const_pool.tile([128, HF, S], BF16)      # [p=f, hf, s]

    wq_r = wq_d.rearrange("h (et p) f -> h p et f", p=128)
    wk_r = wk_d.rearrange("h (et p) f -> h p et f", p=128)
    wv_r = wv_d.rearrange("h (et p) f -> h p et f", p=128)

    for h in range(H):
        wq_sb = w_pool.tile([128, ET, E], BF16)
        wk_sb = w_pool.tile([128, ET, E], BF16)
        wv_sb = w_pool.tile([128, ET, E], BF16)
        if h == 0:
            # Only a hw queue's FIRST dma streams fast. sync#1 = exactly what
            # the first q-group needs (xT1 + wq ft0), sized to finish before
            # scalar's first packet (~10.5us) so the queues never overlap and
            # both run at full rate. scalar#1 = wq ft1-3 (needed ~12.9us).
            # scalar#2 (xT2) crawls but Phase A is reordered q-sc0, k-sc0,
            # q-sc1 so xT2 isn't needed until ~26us.
            nc.sync.dma_start(wx_sb[:, :, 0:640], wx_r[:, :, 0:640])
            nc.scalar.dma_start(wx_sb[:, :, 640:1024], wx_r[:, :, 640:1024])
            nc.scalar.dma_start(wx_sb[:, :, 1024:1536], wx_r[:, :, 1024:1536])
            nc.gpsimd.dma_start(wk_sb[:], wk_r[0])
            nc.gpsimd.dma_start(wv_sb[:], wv_r[0])
            nc.gpsimd.dma_start(wo_sb[:], wo_d.rearrange("(hf p) e -> p hf e", p=128))
            nc.gpsimd.memset(ones8_sb[:], 1.0)
        else:
            nc.gpsimd.dma_start(wq_sb[:], wq_r[h])
            nc.gpsimd.dma_start(wk_sb[:], wk_r[h])
            nc.gpsimd.dma_start(wv_sb[:], wv_r[h])

        qT_sb = act_pool.tile([128, ET, S], BF16)     # [p=f, ft, s]
        kT_sb = act_pool.tile([128, ET, S], BF16)
        v_sb = act_pool.tile([128, TT, E], BF16)      # [p=t, tt, f]
        expE_sb = act_pool.tile([128, TT, S], BF16)   # [p=t, tt, s]
        expE8_sb = act_pool.tile([128, TT, S], FP8)   # fp8 copy for denominator
        recip_sb = act_pool.tile([128, SC, 512], F32)

        # q/k projections -> [f, s]; xT lives in wx cols 0:512 (sc0) and
        # 1024:1536 (sc1); head-0 Wq is packed into wx cols 512:1024
        qw = (wx_sb, 512) if h == 0 else (wq_sb, 0)
        for sc in range(SC):
            for (w_t, w_off), dst in ((qw, qT_sb), ((wk_sb, 0), kT_sb)):
                for ft in range(ET):
                    ps = psum_pool.tile([128, 512], F32)
                    for et in range(ET):
                        nc.tensor.matmul(
                            ps[:],
                            w_t[:, et, w_off + ft * 128:w_off + (ft + 1) * 128],
                            wx_sb[:, et, sc * 1024:sc * 1024 + 512],
                            start=(et == 0), stop=(et == ET - 1),
                        )
                    nc.scalar.activation(
                        dst[:, ft, sc * 512:(sc + 1) * 512], ps[:], AF.Copy)

        # v projection -> [t, f]
        for tt in range(TT):
            xo = tt * 128 if tt < 4 else 1024 + (tt - 4) * 128
            ps = psum_pool.tile([128, 512], F32)
            for et in range(ET):
                nc.tensor.matmul(
                    ps[:],
                    wx_sb[:, et, xo:xo + 128],
                    wv_sb[:, et, :],
                    start=(et == 0), stop=(et == ET - 1),
                )
            nc.scalar.activation(v_sb[:, tt, :], ps[:], AF.Copy)

        # scoresT + fused exp(scale*scores), sc-outer so each sc's
        # denominator (fp8 DoubleRow ones-matmul, 2x PE throughput) and
        # reciprocal pipeline into the middle of the phase
        for sc in range(SC):
            for tt in range(TT):
                ps = psum_pool.tile([128, 512], F32)
                for ft in range(ET):
                    nc.tensor.matmul(
                        ps[:],
                        kT_sb[:, ft, tt * 128:(tt + 1) * 128],
                        qT_sb[:, ft, sc * 512:(sc + 1) * 512],
                        start=(ft == 0), stop=(ft == ET - 1),
                    )
                nc.scalar.activation(
                    expE_sb[:, tt, sc * 512:(sc + 1) * 512], ps[:],
                    AF.Exp, scale=SCALE)
                nc.vector.tensor_copy(
                    expE8_sb[:, tt, sc * 512:(sc + 1) * 512],
                    expE_sb[:, tt, sc * 512:(sc + 1) * 512])
            ps = psum_pool.tile([128, 512], F32)
            for tt in range(0, TT, 2):
                nc.tensor.matmul(
                    ps[:], ones8_sb[:, 0:2, :],
                    expE8_sb[:, tt:tt + 2, sc * 512:(sc + 1) * 512],
                    start=(tt == 0), stop=(tt == TT - 2),
                    perf_mode=mybir.MatmulPerfMode.DoubleRow,
                )
            nc.vector.reciprocal(recip_sb[:, sc, :], ps[:])

        # oT = v.T @ expE, normalized; sc-outer so sc0's tensor_muls drain
        # while sc1's reciprocal is still in flight
        for sc in range(SC):
            for ft in range(ET):
                ps = psum_pool.tile([128, 512], F32)
                for tt in range(TT):
                    nc.tensor.matmul(
                        ps[:],
                        v_sb[:, tt, ft * 128:(ft + 1) * 128],
                        expE_sb[:, tt, sc * 512:(sc + 1) * 512],
                        start=(tt == 0), stop=(tt == TT - 1),
                    )
                nc.vector.tensor_mul(
                    oT_all[:, h * ET + ft, sc * 512:(sc + 1) * 512],
                    ps[:], recip_sb[:, sc, :])

    # output projection: out[s, e] = sum_f o_concat[s, f] Wo[f, e]
    out_r = out_d.rearrange("(st p) e -> p st e", p=128)
    for st in range(TT):
        ps = psum_pool.tile([128, 512], F32)
        for hf in range(HF):
            nc.tensor.matmul(
                ps[:],
                oT_all[:, hf, st * 128:(st + 1) * 128],
                wo_sb[:, hf, :],
                start=(hf == 0), stop=(hf == HF - 1),
            )
        o_sb = out_pool.tile([128, 512], F32)
        nc.vector.tensor_copy(o_sb[:], ps[:])
        nc.sync.dma_start(out_r[:, st, :], o_sb[:])


def _build():
    nc = bacc.Bacc("TRN2", target_bir_lowering=False, debug=False,
                   enable_asserts=False, num_devices=B)
    wx_d = nc.dram_tensor("wx", [E, 1536], BF16, kind="ExternalInput").ap()
    wq_d = nc.dram_tensor("wq", [H, E, E], BF16, kind="ExternalInput").ap()
    wk_d = nc.dram_tensor("wk", [H, E, E], BF16, kind="ExternalInput").ap()
    wv_d = nc.dram_tensor("wv", [H, E, E], BF16, kind="ExternalInput").ap()
    wo_d = nc.dram_tensor("wo", [H * E, E], BF16, kind="ExternalInput").ap()
    out_d = nc.dram_tensor("out", [S, E], F32, kind="ExternalOutput").ap()

    with tile.TileContext(nc) as tc, ExitStack() as ctx:
        _emit(ctx, tc, wx_d, wq_d, wk_d, wv_d, wo_d, out_d)
    nc.compile()
    return nc


def kernel(x, Wq, Wk, Wv, Wo, **_unused_zero_biases):
    global _compiled_nc, last_exec_time_ns
    if _compiled_nc is None:
        _compiled_nc = _build()

    bf = ml_dtypes.bfloat16
    x = np.asarray(x)
    wq_np = np.asarray(Wq).astype(bf)
    wk_np = np.asarray(Wk).astype(bf)
    wv_np = np.asarray(Wv).astype(bf)
    wo_np = np.asarray(Wo).astype(bf)
    in_maps = []
    for b in range(B):
        xTb = x[b].T.astype(bf)
        wx = np.concatenate([xTb[:, 0:512], wq_np[0], xTb[:, 512:1024]], axis=1)
        in_maps.append({"wx": wx, "wq": wq_np, "wk": wk_np,
                        "wv": wv_np, "wo": wo_np})
    trace = bool(int(os.environ.get("KERNEL_TRACE", "0")))
    res = bass_utils.run_bass_kernel_spmd(
        _compiled_nc, in_maps, core_ids=list(range(B)), trace=trace)
    last_exec_time_ns = res.exec_time_ns
    return np.stack([res.results[b]["out"] for b in range(B)], axis=0)
